# revision 1
# baseline (speedup 1.0000x reference)
"""Trainium2 Bass kernel for nn_KernelMachine (random Fourier features).

out[n,m] = sum_f sqrt(2/F) * cos(x_n . a_f + b_f) * W[f*M+m]

Strategy (data-parallel over 8 NeuronCores, N sharded, a/b/W replicated):

Per core (N_loc=4096, D=16, F=4096, M=16):
  1. m1 (PE, bf16 split):  t = (x @ a.T + b + pi/2) / (2*pi)  in PSUM fp32.
     x and a are split hi/lo in bf16 (3-term product) so t is accurate to
     ~2^-18; b (+pi/2, /2pi) rides as two extra contraction rows against
     ones-rows. K=50, one pass, full speed.
  2. DVE magic-round:      k = (t + 1.5*2^23) - 1.5*2^23  (exact rint), bf16.
  3. corr (PE):            t -= I @ k  accumulated into the same PSUM bank,
     giving s = t - rint(t) in [-0.5, 0.5]  (exact Sterbenz subtraction).
  4. ACT:                  phi = Sin(2*pi*s)  == cos(x.a + b), fp32(r) SBUF.
     (ScalarE Sin is only valid on [-pi, pi]; the mod-1 range reduction above
     makes the argument exact up to t's error.)
  5. m2 (PE, f32r):        outT[m, n] += (W*sqrt(2/F))[f,m].T @ phi[f, n]
     accumulated over the 32 f-chunks. f32r moving at N=512 is full rate.
  6. epilogue: PE-transpose outT [16,512] -> [512,16] and DMA out.
"""

import math

import numpy as np
import ml_dtypes

import concourse.bass as bass
import concourse.tile as tile
from concourse import bacc, mybir
from concourse.bass_utils import run_bass_kernel_spmd

F32 = mybir.dt.float32
F32R = mybir.dt.float32r
BF16 = mybir.dt.bfloat16
FP16 = mybir.dt.float16

N, D, F, M = 32768, 16, 4096, 16
NCORES = 8
NLOC = N // NCORES            # 4096 rows per core
FC = F // 128                 # 32 f-chunks of 128
NJ = NLOC // 512              # 8 n-chunks of 512

MAGIC = float(np.float32(1.5 * 2 ** 23))
M2_FP16 = False  # f32r m2: measured more accurate AND as fast as fp16
TWO_PI = float(2.0 * np.pi)

PHI_DT = FP16 if M2_FP16 else F32R
WSC_DT = FP16 if M2_FP16 else F32R
W_PRESCALE = 256.0 if M2_FP16 else 1.0

_CACHE = {}


def build_nc():
    nc = bacc.Bacc(None, target_bir_lowering=False)

    x_in = nc.dram_tensor("x_in", [NLOC, D], F32, kind="ExternalInput")
    apack_in = nc.dram_tensor("apack_in", [50, F], BF16, kind="ExternalInput")
    wsc_in = nc.dram_tensor("wsc_in", [128, FC, M], WSC_DT, kind="ExternalInput")
    negi_in = nc.dram_tensor("negi_in", [128, 128], BF16, kind="ExternalInput")
    ident_in = nc.dram_tensor("ident_in", [128, 128], F32, kind="ExternalInput")
    ones_in = nc.dram_tensor("ones_in", [2, NLOC], BF16, kind="ExternalInput")
    out_t = nc.dram_tensor("out", [NLOC, M], F32, kind="ExternalOutput")

    with tile.TileContext(nc) as tc:
        with (
            tc.tile_pool(name="const", bufs=1) as const,
            tc.tile_pool(name="kp", bufs=3) as kp,
            tc.tile_pool(name="php", bufs=5) as php,
            tc.tile_pool(name="osb", bufs=2) as osb,
            tc.tile_pool(name="pst", bufs=3, space="PSUM") as pst,
            tc.tile_pool(name="pso", bufs=2, space="PSUM") as pso,
        ):
            # ---------------- constants ----------------
            apack = const.tile([50, F], BF16, tag="apack")
            nc.sync.dma_start(out=apack, in_=apack_in[:])
            wsc = const.tile([128, FC, M], WSC_DT, tag="wsc")
            nc.sync.dma_start(out=wsc, in_=wsc_in[:])
            negi = const.tile([128, 128], BF16, tag="negi")
            nc.sync.dma_start(out=negi, in_=negi_in[:])
            ident = const.tile([128, 128], F32, tag="ident")
            nc.sync.dma_start(out=ident, in_=ident_in[:])

            # ---------------- x prologue ----------------
            # xf[p, c, d] = x[128c + p, d].  xcomb[:, c, 0:16] = hi split,
            # xcomb[:, c, 16:32] = residual, so ONE [128,32] PE transpose per
            # chunk yields both xh^T and xl^T rows (psum [32,128]); one ACT
            # copy per group moves both to a staging tile and DMAs fan the
            # rows out to xpack.  All chunked per 512-column group g so main
            # loop chunk j=g starts while later groups still load.
            xf = const.tile([128, FC, D], F32, tag="xf")
            xh_b = const.tile([128, FC, D], BF16, tag="xhb")
            xcomb = const.tile([128, FC, 2 * D], F32, tag="xcomb")
            x_re = x_in[:].rearrange("(c p) d -> p c d", p=128)

            # xpack rows: [xh(0:16), xl(16:32), xh(32:48), ones(48:50)]
            xpack = const.tile([50, NLOC], BF16, tag="xpack")
            stg = const.tile([32, NLOC], BF16, tag="stg")

            def emit_xgroup(g):
                # The PE queue is strictly in-order, so these transposes are
                # interleaved into the main loop (group g emitted ~2
                # iterations before its consumers) instead of all up front.
                sl = slice(4 * g, 4 * (g + 1))
                nc.sync.dma_start(out=xf[:, sl, :], in_=x_re[:, sl, :])
                nc.vector.tensor_copy(out=xh_b[:, sl, :], in_=xf[:, sl, :])
                nc.vector.tensor_copy(out=xcomb[:, sl, 0:D], in_=xh_b[:, sl, :])
                nc.vector.tensor_tensor(
                    out=xcomb[:, sl, D:2 * D], in0=xf[:, sl, :],
                    in1=xcomb[:, sl, 0:D], op=mybir.AluOpType.subtract,
                )
                tpc = pst.tile([32, 512], F32, tag="t")
                for q in range(4):
                    c = 4 * g + q
                    nc.tensor.transpose(
                        tpc[:, 128 * q:128 * (q + 1)], xcomb[:, c, :], ident
                    )
                nc.scalar.copy(out=stg[:, 512 * g:512 * (g + 1)], in_=tpc)
                cols = slice(512 * g, 512 * (g + 1))
                nc.sync.dma_start(out=xpack[0:16, cols], in_=stg[0:16, cols])
                nc.sync.dma_start(out=xpack[16:32, cols], in_=stg[16:32, cols])
                nc.sync.dma_start(out=xpack[32:48, cols], in_=stg[0:16, cols])

            nc.sync.dma_start(out=xpack[48:50, :], in_=ones_in[:])
            for _g in range(NJ):
                emit_xgroup(_g)

            # ---------------- main loop (software-pipelined) ----------------
            # Iterate cp = pair-of-f-chunks (tile [128,1024] = 2 banks).
            # Emission order per iteration keeps every PE consumer >=1.5
            # iterations behind its cross-engine producer:
            #   PE:  m1(cp) | m2(cp-2) | corr(cp-1)
            #   DVE: k halves of cp (after m1 halves)
            #   ACT: sin(cp-1) (after corr(cp-1))
            NIT = NJ * (FC // 2)           # 128 iterations, 2 chunks each
            t_tiles = {}
            k_tiles = {}
            phi_tiles = {}
            out_ps_by_j = {}

            def emit_epilogue(j):
                out_ps = out_ps_by_j.pop(j)
                outT = osb.tile([32, 512], F32, tag="outT")
                nc.gpsimd.memset(outT, 0.0)
                nc.scalar.mul(outT[0:16, :], out_ps, 1.0 / W_PRESCALE)
                blockT = osb.tile([32, 512], F32, tag="blockT")
                nc.vector.transpose(out=blockT, in_=outT)
                nc.sync.dma_start(
                    out=out_t[512 * j:512 * (j + 1), :].rearrange(
                        "(cb i) m -> i cb m", i=32
                    ),
                    in_=blockT.rearrange("p (cb jj) -> p cb jj", jj=32)[:, :, 0:M],
                )

            for it in range(NIT + 3):
                # ---- m1(it) + k(it) ----
                if it < NIT:
                    j, cp = divmod(it, FC // 2)
                    tp = pst.tile([128, 1024], F32, tag="t")
                    for h in range(2):
                        c = 2 * cp + h
                        nc.tensor.matmul(
                            tp[:, 512 * h:512 * (h + 1)],
                            apack[:, 128 * c:128 * (c + 1)],
                            xpack[:, 512 * j:512 * (j + 1)],
                            start=True, stop=False,
                        )
                    t_tiles[it] = tp
                    k_bf = kp.tile([128, 1024], BF16, tag="k")
                    nc.vector.tensor_scalar(
                        out=k_bf, in0=tp,
                        scalar1=MAGIC, scalar2=MAGIC,
                        op0=mybir.AluOpType.add, op1=mybir.AluOpType.subtract,
                    )
                    k_tiles[it] = k_bf
                # ---- m2(it-3) ----
                if 0 <= it - 3 < NIT:
                    it2 = it - 3
                    j2, cp2 = divmod(it2, FC // 2)
                    if cp2 == 0:
                        out_ps = pso.tile([16, 512], F32, tag="o")
                        out_ps_by_j[j2] = out_ps
                    out_ps = out_ps_by_j[j2]
                    phi = phi_tiles.pop(it2)
                    for h in range(2):
                        c = 2 * cp2 + h
                        nc.tensor.matmul(
                            out_ps,
                            wsc[:, c, :],
                            phi[:, 512 * h:512 * (h + 1)],
                            start=(c == 0), stop=(c == FC - 1),
                        )
                    if cp2 == FC // 2 - 1:
                        emit_epilogue(j2)
                # ---- corr(it-1) + sin(it-1) ----
                if 0 <= it - 1 < NIT:
                    it1 = it - 1
                    tp = t_tiles.pop(it1)
                    k_bf = k_tiles.pop(it1)
                    for h in range(2):
                        nc.tensor.matmul(
                            tp[:, 512 * h:512 * (h + 1)],
                            negi,
                            k_bf[:, 512 * h:512 * (h + 1)],
                            start=False, stop=True,
                        )
                    phi = php.tile([128, 1024], PHI_DT, tag="phi")
                    nc.scalar.activation(
                        out=phi, in_=tp,
                        func=mybir.ActivationFunctionType.Sin,
                        bias=0.0, scale=TWO_PI,
                    )
                    phi_tiles[it1] = phi

    nc.finalize()
    return nc


def _host_prep(a, b, W):
    """Precompute replicated operand packs (float64 for exact splitting)."""
    inv2pi = 1.0 / (2.0 * np.pi)
    a64 = np.asarray(a, dtype=np.float64).T * inv2pi          # [16, F]
    ah = a64.astype(ml_dtypes.bfloat16)
    al = (a64 - ah.astype(np.float64)).astype(ml_dtypes.bfloat16)
    b64 = (np.asarray(b, dtype=np.float64) + np.pi / 2.0) * inv2pi  # [F]
    bh = b64.astype(ml_dtypes.bfloat16)
    bl = (b64 - bh.astype(np.float64)).astype(ml_dtypes.bfloat16)

    apack = np.zeros((50, F), dtype=ml_dtypes.bfloat16)
    apack[0:16] = ah       # pairs with xh
    apack[16:32] = ah      # pairs with xl
    apack[32:48] = al      # pairs with xh (dup rows)
    apack[48] = bh
    apack[49] = bl

    scale = math.sqrt(2.0 / F)
    wdt = np.float16 if M2_FP16 else np.float32
    W2 = (np.asarray(W, dtype=np.float64).reshape(F, M) * scale * W_PRESCALE).astype(wdt)
    wsc = np.ascontiguousarray(
        W2.reshape(FC, 128, M).transpose(1, 0, 2)
    )                                                          # [128, FC, M]

    negi = (-np.eye(128)).astype(ml_dtypes.bfloat16)
    ident = np.eye(128, dtype=np.float32)
    ones = np.ones((2, NLOC), dtype=ml_dtypes.bfloat16)
    return apack, wsc, negi, ident, ones


def kernel(x, a, b, W):
    x = np.ascontiguousarray(np.asarray(x, dtype=np.float32))
    apack, wsc, negi, ident, ones = _host_prep(a, b, W)

    if "nc" not in _CACHE:
        _CACHE["nc"] = build_nc()
    nc = _CACHE["nc"]

    in_maps = []
    for i in range(NCORES):
        in_maps.append({
            "x_in": np.ascontiguousarray(x[i * NLOC:(i + 1) * NLOC]),
            "apack_in": apack,
            "wsc_in": wsc,
            "negi_in": negi,
            "ident_in": ident,
            "ones_in": ones,
        })

    res = run_bass_kernel_spmd(nc, in_maps, core_ids=list(range(NCORES)))
    return np.concatenate([r["out"] for r in res.results], axis=0)



# revision 7
# speedup vs baseline: 1.1832x; 1.1832x over previous
"""Trainium2 Bass kernel for nn_KernelMachine (random Fourier features).

out[n,m] = sum_f sqrt(2/F) * cos(x_n . a_f + b_f) * W[f*M+m]

Strategy (data-parallel over 8 NeuronCores, N sharded, a/b/W replicated):

Host prep: x is split hi/lo in bf16 and packed (transposed) into xpack rows
on the host, so the device runs only the main pipeline. The 50 contraction
rows (ah.xh + ah.xl + al.xh + bias rows) are duplicated at partition offset
64 so the two f-chunks of each iteration run as CONCURRENT row-tiled
matmuls on disjoint PE row-groups.

Per core (N_loc=4096, D=16, F=4096, M=16), per iteration (2 f-chunks x 512 n):
  1. m1 (PE, row-tiled pair): t = (x @ a.T + b + pi/2) / (2*pi) in PSUM fp32.
     tile_position (0,0) and (64,0) -> both 512-col matmuls overlap (~250ns).
  2. DVE magic-round: k = (t + 1.5*2^23) - 1.5*2^23 (exact rint), bf16.
     (PSUM fp32 source = 1x mode; this is the pipeline bottleneck engine.)
  3. corr (PE): t -= I @ k accumulated into the same PSUM bank, giving
     s = t - rint(t) in [-0.5, 0.5] (exact Sterbenz subtraction).
  4. ACT: phi = Sin(2*pi*s) == cos(x.a + b), f32r SBUF.
  5. m2 (PE, f32r): outT[m, n] += (W*sqrt(2/F))[f,m].T @ phi[f, n]
     accumulated over the 32 f-chunks.
  6. epilogue per 512-row group: ACT copies outT [16,512] PSUM->SBUF fp32,
     straight DMA into out[16, N_loc]; the host transposes to [N_loc, 16].
     (Keeps the DVE, which is the critical engine, out of the epilogue.)
"""

import math

import numpy as np
import ml_dtypes

import concourse.bass as bass
import concourse.tile as tile
from concourse import bacc, mybir
from concourse.bass_utils import run_bass_kernel_spmd

F32 = mybir.dt.float32
F32R = mybir.dt.float32r
BF16 = mybir.dt.bfloat16
FP16 = mybir.dt.float16

N, D, F, M = 32768, 16, 4096, 16
NCORES = 8
NLOC = N // NCORES            # 4096 rows per core
FC = F // 128                 # 32 f-chunks of 128
NJ = NLOC // 512              # 8 n-groups of 512
NIT = NJ * (FC // 2)          # 128 iterations, 2 f-chunks each

MAGIC = float(np.float32(1.5 * 2 ** 23))
TWO_PI = float(2.0 * np.pi)
ROW_TILE_M1 = True

_CACHE = {}


def build_nc():
    nc = bacc.Bacc(None, target_bir_lowering=False)

    xpack_in = nc.dram_tensor("xpack_in", [128, NLOC], BF16, kind="ExternalInput")
    apack_in = nc.dram_tensor("apack_in", [128, (FC // 2) * 128], BF16, kind="ExternalInput")
    wsc_in = nc.dram_tensor("wsc_in", [128, FC, M], F32R, kind="ExternalInput")
    negi_in = nc.dram_tensor("negi_in", [128, 128], BF16, kind="ExternalInput")
    out_t = nc.dram_tensor("out", [M, NLOC], F32, kind="ExternalOutput")

    with tile.TileContext(nc) as tc:
        with (
            tc.tile_pool(name="const", bufs=1) as const,
            tc.tile_pool(name="kp", bufs=3) as kp,
            tc.tile_pool(name="php", bufs=5) as php,
            tc.tile_pool(name="osb", bufs=2) as osb,
            tc.tile_pool(name="pst", bufs=3, space="PSUM") as pst,
            tc.tile_pool(name="pso", bufs=2, space="PSUM") as pso,
        ):
            # constants; DMA order puts the first iteration's operands first
            apack = const.tile([128, (FC // 2) * 128], BF16, tag="apack")
            xpack = const.tile([128, NLOC], BF16, tag="xpack")
            negi = const.tile([128, 128], BF16, tag="negi")
            wsc = const.tile([128, FC, M], F32R, tag="wsc")
            nc.sync.dma_start(out=apack[:, 0:512], in_=apack_in[:, 0:512])
            nc.sync.dma_start(out=xpack[:, 0:512], in_=xpack_in[:, 0:512])
            nc.sync.dma_start(out=negi, in_=negi_in[:])
            nc.sync.dma_start(out=apack[:, 512:2048], in_=apack_in[:, 512:2048])
            nc.sync.dma_start(out=xpack[:, 512:NLOC], in_=xpack_in[:, 512:NLOC])
            nc.sync.dma_start(out=wsc, in_=wsc_in[:])

            t_tiles = {}
            k_tiles = {}
            phi_tiles = {}
            out_ps_by_j = {}

            def emit_epilogue(j):
                out_ps = out_ps_by_j.pop(j)
                outT = osb.tile([16, 512], F32, tag="outT")
                nc.scalar.copy(out=outT, in_=out_ps)
                nc.sync.dma_start(
                    out=out_t[:, 512 * j:512 * (j + 1)], in_=outT
                )

            # Software pipeline; per loop step the PE queue receives
            #   m1(it) | m2(it-3) | corr(it-1)
            # so every PE consumer is >=1 step behind its cross-engine producer.
            for it in range(NIT + 3):
                # ---- m1(it) + round(it) ----
                if it < NIT:
                    j, cp = divmod(it, FC // 2)
                    tp = pst.tile([128, 1024], F32, tag="t")
                    blk = slice(128 * cp, 128 * (cp + 1))
                    cols = slice(512 * j, 512 * (j + 1))
                    if ROW_TILE_M1:
                        nc.tensor.matmul(
                            tp[:, 0:512], apack[0:50, blk], xpack[0:50, cols],
                            start=True, stop=False, tile_position=(0, 0),
                        )
                        nc.tensor.matmul(
                            tp[:, 512:1024], apack[64:114, blk], xpack[64:114, cols],
                            start=True, stop=False, tile_position=(64, 0),
                        )
                    else:
                        nc.tensor.matmul(
                            tp[:, 0:512], apack[0:50, blk], xpack[0:50, cols],
                            start=True, stop=False,
                        )
                        nc.tensor.matmul(
                            tp[:, 512:1024], apack[64:114, blk], xpack[64:114, cols],
                            start=True, stop=False,
                        )
                    t_tiles[it] = tp
                    k_bf = kp.tile([128, 1024], BF16, tag="k")
                    nc.vector.tensor_scalar(
                        out=k_bf, in0=tp,
                        scalar1=MAGIC, scalar2=MAGIC,
                        op0=mybir.AluOpType.add, op1=mybir.AluOpType.subtract,
                    )
                    k_tiles[it] = k_bf
                # ---- m2(it-3) ----
                if 0 <= it - 3 < NIT:
                    it2 = it - 3
                    j2, cp2 = divmod(it2, FC // 2)
                    if cp2 == 0:
                        out_ps = pso.tile([16, 512], F32, tag="o")
                        out_ps_by_j[j2] = out_ps
                    out_ps = out_ps_by_j[j2]
                    phi = phi_tiles.pop(it2)
                    for h in range(2):
                        c = 2 * cp2 + h
                        nc.tensor.matmul(
                            out_ps,
                            wsc[:, c, :],
                            phi[:, 512 * h:512 * (h + 1)],
                            start=(c == 0), stop=(c == FC - 1),
                        )
                    if cp2 == FC // 2 - 1:
                        emit_epilogue(j2)
                # ---- corr(it-1) + sin(it-1) ----
                if 0 <= it - 1 < NIT:
                    it1 = it - 1
                    tp = t_tiles.pop(it1)
                    k_bf = k_tiles.pop(it1)
                    for h in range(2):
                        nc.tensor.matmul(
                            tp[:, 512 * h:512 * (h + 1)],
                            negi,
                            k_bf[:, 512 * h:512 * (h + 1)],
                            start=False, stop=True,
                        )
                    phi = php.tile([128, 1024], F32R, tag="phi")
                    nc.scalar.activation(
                        out=phi, in_=tp,
                        func=mybir.ActivationFunctionType.Sin,
                        bias=0.0, scale=TWO_PI,
                    )
                    phi_tiles[it1] = phi

    nc.finalize()
    return nc


def _host_prep(a, b, W):
    """Replicated operand packs (float64 intermediates for exact splitting)."""
    inv2pi = 1.0 / (2.0 * np.pi)
    a64 = np.asarray(a, dtype=np.float64).T * inv2pi          # [16, F]
    ah = a64.astype(ml_dtypes.bfloat16)
    al = (a64 - ah.astype(np.float64)).astype(ml_dtypes.bfloat16)
    b64 = (np.asarray(b, dtype=np.float64) + np.pi / 2.0) * inv2pi  # [F]
    bh = b64.astype(ml_dtypes.bfloat16)
    bl = (b64 - bh.astype(np.float64)).astype(ml_dtypes.bfloat16)

    # apack: 16 column-blocks of 128; block cp holds f-chunk 2cp in rows
    # 0:50 and f-chunk 2cp+1 in rows 64:114 (for the row-tiled m1 pair).
    apack = np.zeros((128, (FC // 2) * 128), dtype=ml_dtypes.bfloat16)
    for half, roff in ((0, 0), (1, 64)):
        ahh = ah.reshape(D, FC, 128)[:, half::2, :].reshape(D, -1)
        all_ = al.reshape(D, FC, 128)[:, half::2, :].reshape(D, -1)
        bhh = bh.reshape(FC, 128)[half::2, :].reshape(-1)
        bll = bl.reshape(FC, 128)[half::2, :].reshape(-1)
        apack[roff + 0:roff + 16] = ahh
        apack[roff + 16:roff + 32] = ahh
        apack[roff + 32:roff + 48] = all_
        apack[roff + 48] = bhh
        apack[roff + 49] = bll

    scale = math.sqrt(2.0 / F)
    W2 = (np.asarray(W, dtype=np.float64).reshape(F, M) * scale).astype(np.float32)
    wsc = np.ascontiguousarray(W2.reshape(FC, 128, M).transpose(1, 0, 2))

    negi = (-np.eye(128)).astype(ml_dtypes.bfloat16)
    return apack, wsc, negi


def _prep_x(x):
    """Full-N xpack [128, N] bf16: hi/lo split rows, duplicated at +64."""
    x64 = np.asarray(x, dtype=np.float64).T                   # [16, N]
    xh = x64.astype(ml_dtypes.bfloat16)
    xl = (x64 - xh.astype(np.float64)).astype(ml_dtypes.bfloat16)
    xpack = np.zeros((128, x64.shape[1]), dtype=ml_dtypes.bfloat16)
    for roff in (0, 64):
        xpack[roff + 0:roff + 16] = xh
        xpack[roff + 16:roff + 32] = xl
        xpack[roff + 32:roff + 48] = xh
        xpack[roff + 48:roff + 50] = 1.0
    return xpack


def make_in_maps(x, a, b, W):
    apack, wsc, negi = _host_prep(a, b, W)
    xpack = _prep_x(x)
    in_maps = []
    for i in range(NCORES):
        in_maps.append({
            "xpack_in": np.ascontiguousarray(xpack[:, i * NLOC:(i + 1) * NLOC]),
            "apack_in": apack,
            "wsc_in": wsc,
            "negi_in": negi,
        })
    return in_maps


def kernel(x, a, b, W):
    if "nc" not in _CACHE:
        _CACHE["nc"] = build_nc()
    nc = _CACHE["nc"]
    in_maps = make_in_maps(x, a, b, W)
    res = run_bass_kernel_spmd(nc, in_maps, core_ids=list(range(NCORES)))
    return np.concatenate(
        [np.ascontiguousarray(np.asarray(r["out"]).T) for r in res.results], axis=0
    )


# revision 9
# speedup vs baseline: 1.1900x; 1.0058x over previous
"""Trainium2 Bass kernel for nn_KernelMachine (random Fourier features).

out[n,m] = sum_f sqrt(2/F) * cos(x_n . a_f + b_f) * W[f*M+m]

Strategy (data-parallel over 8 NeuronCores, N sharded, a/b/W replicated):

Host prep: x is split hi/lo in bf16 and packed (transposed) into xpack rows
on the host, so the device runs only the main pipeline. The 50 contraction
rows (ah.xh + ah.xl + al.xh + bias rows) are duplicated at partition offset
64 so the two f-chunks of each iteration run as CONCURRENT row-tiled
matmuls on disjoint PE row-groups.

Per core (N_loc=4096, D=16, F=4096, M=16), per iteration (2 f-chunks x 512 n):
  1. m1 (PE, row-tiled pair): t = (x @ a.T + b + pi/2) / (2*pi) in PSUM fp32.
     tile_position (0,0) and (64,0) -> both 512-col matmuls overlap (~250ns).
  2. DVE magic-round: k = (t + 1.5*2^23) - 1.5*2^23 (exact rint), bf16.
     (PSUM fp32 source = 1x mode; this is the pipeline bottleneck engine.)
  3. corr (PE): t -= I @ k accumulated into the same PSUM bank, giving
     s = t - rint(t) in [-0.5, 0.5] (exact Sterbenz subtraction).
  4. ACT: phi = Sin(2*pi*s) == cos(x.a + b), f32r SBUF.
  5. m2 (PE, f32r): outT[m, n] += (W*sqrt(2/F))[f,m].T @ phi[f, n]
     accumulated over the 32 f-chunks.
  6. epilogue per 512-row group: ACT copies outT [16,512] PSUM->SBUF fp32,
     straight DMA into out[16, N_loc]; the host transposes to [N_loc, 16].
     (Keeps the DVE, which is the critical engine, out of the epilogue.)
"""

import math

import numpy as np
import ml_dtypes

import concourse.bass as bass
import concourse.tile as tile
from concourse import bacc, mybir
from concourse.bass_utils import run_bass_kernel_spmd

F32 = mybir.dt.float32
F32R = mybir.dt.float32r
BF16 = mybir.dt.bfloat16
FP16 = mybir.dt.float16

N, D, F, M = 32768, 16, 4096, 16
NCORES = 8
NLOC = N // NCORES            # 4096 rows per core
FC = F // 128                 # 32 f-chunks of 128
NJ = NLOC // 512              # 8 n-groups of 512
NIT = NJ * (FC // 2)          # 128 iterations, 2 f-chunks each

MAGIC = float(np.float32(1.5 * 2 ** 23))
TWO_PI = float(2.0 * np.pi)
ROW_TILE_M1 = True

_CACHE = {}


def build_nc():
    nc = bacc.Bacc(None, target_bir_lowering=False)

    xpack_in = nc.dram_tensor("xpack_in", [128, NLOC], BF16, kind="ExternalInput")
    apack_in = nc.dram_tensor("apack_in", [128, (FC // 2) * 128], BF16, kind="ExternalInput")
    wsc_in = nc.dram_tensor("wsc_in", [128, FC, M], F32R, kind="ExternalInput")
    negi_in = nc.dram_tensor("negi_in", [128, 128], BF16, kind="ExternalInput")
    out_t = nc.dram_tensor("out", [M, NLOC], F32, kind="ExternalOutput")

    with tile.TileContext(nc) as tc:
        with (
            tc.tile_pool(name="const", bufs=1) as const,
            tc.tile_pool(name="kp", bufs=6) as kp,
            tc.tile_pool(name="php", bufs=7) as php,
            tc.tile_pool(name="osb", bufs=2) as osb,
            tc.tile_pool(name="pst", bufs=3, space="PSUM") as pst,
            tc.tile_pool(name="pso", bufs=2, space="PSUM") as pso,
        ):
            # constants; DMA order puts the first iteration's operands first
            apack = const.tile([128, (FC // 2) * 128], BF16, tag="apack")
            xpack = const.tile([128, NLOC], BF16, tag="xpack")
            negi = const.tile([128, 128], BF16, tag="negi")
            wsc = const.tile([128, FC, M], F32R, tag="wsc")
            nc.sync.dma_start(out=apack[:, 0:128], in_=apack_in[:, 0:128])
            nc.sync.dma_start(out=xpack[:, 0:512], in_=xpack_in[:, 0:512])
            nc.sync.dma_start(out=negi, in_=negi_in[:])
            nc.sync.dma_start(out=apack[:, 128:512], in_=apack_in[:, 128:512])
            nc.sync.dma_start(out=apack[:, 512:2048], in_=apack_in[:, 512:2048])
            nc.sync.dma_start(out=wsc, in_=wsc_in[:])
            nc.sync.dma_start(out=xpack[:, 512:NLOC], in_=xpack_in[:, 512:NLOC])

            t_tiles = {}
            k_tiles = {}
            phi_tiles = {}
            out_ps_by_j = {}

            def emit_epilogue(j):
                out_ps = out_ps_by_j.pop(j)
                outT = osb.tile([16, 512], F32, tag="outT")
                nc.scalar.copy(out=outT, in_=out_ps)
                nc.sync.dma_start(
                    out=out_t[:, 512 * j:512 * (j + 1)], in_=outT
                )

            # Software pipeline; per loop step the PE queue receives
            #   m1(it) | m2(it-3) | corr(it-1)
            # so every PE consumer is >=1 step behind its cross-engine producer.
            for it in range(NIT + 3):
                # ---- m1(it) + round(it) ----
                if it < NIT:
                    j, cp = divmod(it, FC // 2)
                    tp = pst.tile([128, 1024], F32, tag="t")
                    blk = slice(128 * cp, 128 * (cp + 1))
                    cols = slice(512 * j, 512 * (j + 1))
                    if ROW_TILE_M1:
                        nc.tensor.matmul(
                            tp[:, 0:512], apack[0:50, blk], xpack[0:50, cols],
                            start=True, stop=False, tile_position=(0, 0),
                        )
                        nc.tensor.matmul(
                            tp[:, 512:1024], apack[64:114, blk], xpack[64:114, cols],
                            start=True, stop=False, tile_position=(64, 0),
                        )
                    else:
                        nc.tensor.matmul(
                            tp[:, 0:512], apack[0:50, blk], xpack[0:50, cols],
                            start=True, stop=False,
                        )
                        nc.tensor.matmul(
                            tp[:, 512:1024], apack[64:114, blk], xpack[64:114, cols],
                            start=True, stop=False,
                        )
                    t_tiles[it] = tp
                    k_bf = kp.tile([128, 1024], BF16, tag="k")
                    nc.vector.tensor_scalar(
                        out=k_bf, in0=tp,
                        scalar1=MAGIC, scalar2=MAGIC,
                        op0=mybir.AluOpType.add, op1=mybir.AluOpType.subtract,
                    )
                    k_tiles[it] = k_bf
                # ---- m2(it-3) ----
                if 0 <= it - 3 < NIT:
                    it2 = it - 3
                    j2, cp2 = divmod(it2, FC // 2)
                    if cp2 == 0:
                        out_ps = pso.tile([16, 512], F32, tag="o")
                        out_ps_by_j[j2] = out_ps
                    out_ps = out_ps_by_j[j2]
                    phi = phi_tiles.pop(it2)
                    for h in range(2):
                        c = 2 * cp2 + h
                        nc.tensor.matmul(
                            out_ps,
                            wsc[:, c, :],
                            phi[:, 512 * h:512 * (h + 1)],
                            start=(c == 0), stop=(c == FC - 1),
                        )
                    if cp2 == FC // 2 - 1:
                        emit_epilogue(j2)
                # ---- corr(it-1) + sin(it-1) ----
                if 0 <= it - 1 < NIT:
                    it1 = it - 1
                    tp = t_tiles.pop(it1)
                    k_bf = k_tiles.pop(it1)
                    for h in range(2):
                        nc.tensor.matmul(
                            tp[:, 512 * h:512 * (h + 1)],
                            negi,
                            k_bf[:, 512 * h:512 * (h + 1)],
                            start=False, stop=True,
                        )
                    phi = php.tile([128, 1024], F32R, tag="phi")
                    nc.scalar.activation(
                        out=phi, in_=tp,
                        func=mybir.ActivationFunctionType.Sin,
                        bias=0.0, scale=TWO_PI,
                    )
                    phi_tiles[it1] = phi

    nc.finalize()
    return nc


def _host_prep(a, b, W):
    """Replicated operand packs (float64 intermediates for exact splitting)."""
    inv2pi = 1.0 / (2.0 * np.pi)
    a64 = np.asarray(a, dtype=np.float64).T * inv2pi          # [16, F]
    ah = a64.astype(ml_dtypes.bfloat16)
    al = (a64 - ah.astype(np.float64)).astype(ml_dtypes.bfloat16)
    b64 = (np.asarray(b, dtype=np.float64) + np.pi / 2.0) * inv2pi  # [F]
    bh = b64.astype(ml_dtypes.bfloat16)
    bl = (b64 - bh.astype(np.float64)).astype(ml_dtypes.bfloat16)

    # apack: 16 column-blocks of 128; block cp holds f-chunk 2cp in rows
    # 0:50 and f-chunk 2cp+1 in rows 64:114 (for the row-tiled m1 pair).
    apack = np.zeros((128, (FC // 2) * 128), dtype=ml_dtypes.bfloat16)
    for half, roff in ((0, 0), (1, 64)):
        ahh = ah.reshape(D, FC, 128)[:, half::2, :].reshape(D, -1)
        all_ = al.reshape(D, FC, 128)[:, half::2, :].reshape(D, -1)
        bhh = bh.reshape(FC, 128)[half::2, :].reshape(-1)
        bll = bl.reshape(FC, 128)[half::2, :].reshape(-1)
        apack[roff + 0:roff + 16] = ahh
        apack[roff + 16:roff + 32] = ahh
        apack[roff + 32:roff + 48] = all_
        apack[roff + 48] = bhh
        apack[roff + 49] = bll

    scale = math.sqrt(2.0 / F)
    W2 = (np.asarray(W, dtype=np.float64).reshape(F, M) * scale).astype(np.float32)
    wsc = np.ascontiguousarray(W2.reshape(FC, 128, M).transpose(1, 0, 2))

    negi = (-np.eye(128)).astype(ml_dtypes.bfloat16)
    return apack, wsc, negi


def _prep_x(x):
    """Full-N xpack [128, N] bf16: hi/lo split rows, duplicated at +64."""
    x64 = np.asarray(x, dtype=np.float64).T                   # [16, N]
    xh = x64.astype(ml_dtypes.bfloat16)
    xl = (x64 - xh.astype(np.float64)).astype(ml_dtypes.bfloat16)
    xpack = np.zeros((128, x64.shape[1]), dtype=ml_dtypes.bfloat16)
    for roff in (0, 64):
        xpack[roff + 0:roff + 16] = xh
        xpack[roff + 16:roff + 32] = xl
        xpack[roff + 32:roff + 48] = xh
        xpack[roff + 48:roff + 50] = 1.0
    return xpack


def make_in_maps(x, a, b, W):
    apack, wsc, negi = _host_prep(a, b, W)
    xpack = _prep_x(x)
    in_maps = []
    for i in range(NCORES):
        in_maps.append({
            "xpack_in": np.ascontiguousarray(xpack[:, i * NLOC:(i + 1) * NLOC]),
            "apack_in": apack,
            "wsc_in": wsc,
            "negi_in": negi,
        })
    return in_maps


def kernel(x, a, b, W):
    if "nc" not in _CACHE:
        _CACHE["nc"] = build_nc()
    nc = _CACHE["nc"]
    in_maps = make_in_maps(x, a, b, W)
    res = run_bass_kernel_spmd(nc, in_maps, core_ids=list(range(NCORES)))
    return np.concatenate(
        [np.ascontiguousarray(np.asarray(r["out"]).T) for r in res.results], axis=0
    )
